# revision 1
# baseline (speedup 1.0000x reference)
"""Trainium2 Bass kernel for nn_DefuzzyLayer: out = x @ rules_outcome.

x: [8192, 4096] f32, rules_outcome: [4096, 4096] f32 -> out: [8192, 4096] f32.

Strategy: data-parallel over batch. Each of the 8 NeuronCores computes a
[1024, 4096] output shard: x_shard @ W with the full W replicated.

Per-core kernel (Tile framework):
  - x shard is staged host-side as x^T [4096, 1024] so the contraction dim
    lands on SBUF partitions; it stays fully resident in SBUF (128 KiB/part).
  - W streams through a ring of [128, 512] SBUF tiles, one pass total.
  - Matmuls run in float32r (FP22 reduced-precision fp32): full PE rate at
    free-dim >= 256, ~1e-4 relative error. Accumulation is fp32 in PSUM.
  - Loop: n-block (512 cols) outer, k inner, m innermost; each n-block
    accumulates 8 m-tiles into the 8 PSUM banks over 32 k-tiles, then
    evicts PSUM -> SBUF -> DRAM.
"""

import numpy as np

BATCH = 8192
IN_DIM = 4096
OUT_DIM = 4096
N_CORES = 8
M_SHARD = BATCH // N_CORES  # 1024

P = 128
NB = 512                    # n-block width (PSUM bank = 512 fp32)
KT = IN_DIM // P            # 32 k-tiles
MT = M_SHARD // P           # 8 m-tiles
NBLK = OUT_DIM // NB        # 8 n-blocks
W_BUFS = 16                 # W tile ring slots (2 KiB/partition each)

_cached_nc = None


def _build():
    import concourse.bacc as bacc
    import concourse.tile as tile
    import concourse.mybir as mybir

    nc = bacc.Bacc("TRN2", target_bir_lowering=False, debug=False)
    xt = nc.dram_tensor(
        "xt", [IN_DIM, M_SHARD], mybir.dt.float32r, kind="ExternalInput"
    ).ap()
    w = nc.dram_tensor(
        "w", [IN_DIM, OUT_DIM], mybir.dt.float32r, kind="ExternalInput"
    ).ap()
    out = nc.dram_tensor(
        "out", [M_SHARD, OUT_DIM], mybir.dt.float32, kind="ExternalOutput"
    ).ap()

    with tile.TileContext(nc) as tc:
        with (
            tc.tile_pool(name="xpool", bufs=KT) as xpool,
            tc.tile_pool(name="wpool", bufs=W_BUFS) as wpool,
            tc.tile_pool(name="opool", bufs=4) as opool,
            tc.tile_pool(name="pspool", bufs=8, space="PSUM") as pspool,
        ):
            x_tiles = []
            for k in range(KT):
                x_k = xpool.tile([P, M_SHARD], mybir.dt.float32r,
                                 name=f"x{k}", tag="x")
                nc.sync.dma_start(out=x_k[:], in_=xt[k * P:(k + 1) * P, :])
                x_tiles.append(x_k)

            for b in range(NBLK):
                w_tiles = []
                for k in range(KT):
                    w_k = wpool.tile([P, NB], mybir.dt.float32r,
                                     name=f"w{b}_{k}", tag="w")
                    nc.sync.dma_start(
                        out=w_k[:],
                        in_=w[k * P:(k + 1) * P, b * NB:(b + 1) * NB],
                    )
                    w_tiles.append(w_k)

                psums = []
                for m in range(MT):
                    ps = pspool.tile([P, NB], mybir.dt.float32,
                                     name=f"ps{b}_{m}", tag="ps")
                    psums.append(ps)

                for k in range(KT):
                    for m in range(MT):
                        nc.tensor.matmul(
                            psums[m][:],
                            x_tiles[k][:, m * P:(m + 1) * P],
                            w_tiles[k][:],
                            start=(k == 0),
                            stop=(k == KT - 1),
                        )

                for m in range(MT):
                    o = opool.tile([P, NB], mybir.dt.float32,
                                   name=f"o{b}_{m}", tag="o")
                    nc.vector.tensor_copy(o[:], psums[m][:])
                    nc.sync.dma_start(
                        out=out[m * P:(m + 1) * P, b * NB:(b + 1) * NB],
                        in_=o[:],
                    )

    nc.compile()
    return nc


def _get_nc():
    global _cached_nc
    if _cached_nc is None:
        _cached_nc = _build()
    return _cached_nc


def _run(x, rules_outcome, **spmd_kwargs):
    from concourse.bass_utils import run_bass_kernel_spmd

    x = np.ascontiguousarray(x, dtype=np.float32)
    w = np.ascontiguousarray(rules_outcome, dtype=np.float32)
    assert x.shape == (BATCH, IN_DIM) and w.shape == (IN_DIM, OUT_DIM)

    xT = np.ascontiguousarray(x.T)  # [IN_DIM, BATCH]
    in_maps = [
        {
            "xt": np.ascontiguousarray(xT[:, i * M_SHARD:(i + 1) * M_SHARD]),
            "w": w,
        }
        for i in range(N_CORES)
    ]

    nc = _get_nc()
    res = run_bass_kernel_spmd(nc, in_maps, core_ids=list(range(N_CORES)),
                               **spmd_kwargs)
    full = np.concatenate([res.results[i]["out"] for i in range(N_CORES)],
                          axis=0)
    return full, res


def kernel(x, rules_outcome):
    out, _ = _run(x, rules_outcome)
    return out


# revision 13
# speedup vs baseline: 2.3878x; 2.3878x over previous
"""Trainium2 Bass kernel for nn_DefuzzyLayer: out = x @ rules_outcome.

x: [8192, 4096] f32, rules_outcome: [4096, 4096] f32 -> out: [8192, 4096] f32.

Strategy: data-parallel over batch. Each of the 8 NeuronCores computes a
[1024, 4096] output shard: x_shard @ W with the full W replicated.

Per-core kernel (Tile framework):
  - Inputs are cast to fp16 host-side (PSUM accumulation stays fp32, so the
    only loss is input rounding: ~3e-4 relative error on the output).
  - Host pre-packs x^T and W into partition-major SBUF layout so every load
    is a fully-contiguous multi-MiB DMA (one for each x quarter, one per W
    n-block); stores batch one [1024, 512] block per DMA on the scalar
    HWDGE ring, separate from the load ring on the sync engine.
  - x shard stays fully resident in SBUF (64 KiB/partition in fp16);
    W streams through 2 double-buffered block tiles.
  - Loop: n-block (512 cols) outer, k inner, m innermost; each n-block
    accumulates 8 m-tiles into the 8 PSUM banks over 32 k-tiles, then
    evicts PSUM -> SBUF -> DRAM.
"""

import numpy as np

BATCH = 8192
IN_DIM = 4096
OUT_DIM = 4096
N_CORES = 8
M_SHARD = BATCH // N_CORES  # 1024

P = 128
NB = int(__import__("os").environ.get("KNB", "256"))  # moving free-dim per matmul
KT = IN_DIM // P            # 32 k-tiles
MT = M_SHARD // P           # 8 m-tiles
NBLK = OUT_DIM // NB        # 8 n-blocks
XCHUNKS = int(__import__("os").environ.get("KXC", "8"))  # x load split
KPC = KT // XCHUNKS         # k-tiles per x chunk
ORDER = __import__("os").environ.get("KORDER", "mi")  # "mi": k outer/m inner; "ki": m outer/k inner
PS_BUFS = int(__import__("os").environ.get("KPSBUFS", "8"))

IN_DT = __import__("os").environ.get("KDT", "float16")  # float32r | float16 | bfloat16

_cached_nc = None


def _np_dt():
    if IN_DT == "float16":
        return np.float16
    if IN_DT == "bfloat16":
        import ml_dtypes
        return np.dtype(ml_dtypes.bfloat16)
    return np.float32


def _build(loop_n=1, in_dt=None, variant="full"):
    """Build + compile the per-core Bass module.

    loop_n > 1 wraps the whole body in an on-device For_i loop — used only
    for HW timing (amortizes host dispatch overhead out of the measurement).
    variant: "full" | "nodma" (skip x/w loads) | "dmaonly" (skip compute).
    """
    import contextlib
    import concourse.bacc as bacc
    import concourse.tile as tile
    import concourse.mybir as mybir

    do_in_dma = variant not in ("nodma", "mmonly")
    do_compute = variant != "dmaonly"
    do_evict = variant != "mmonly"

    dt_in = getattr(mybir.dt, in_dt or IN_DT)

    nc = bacc.Bacc("TRN2", target_bir_lowering=False, debug=False)
    # partition-major packed inputs (see _pack_x_shard/_pack_w)
    xt = nc.dram_tensor(
        "xt", [P, KT * M_SHARD], dt_in, kind="ExternalInput"
    ).ap()
    w = nc.dram_tensor(
        "w", [P, NBLK * KT * NB], dt_in, kind="ExternalInput"
    ).ap()
    out = nc.dram_tensor(
        "out", [M_SHARD, OUT_DIM], mybir.dt.float32, kind="ExternalOutput"
    ).ap()
    out_r = out.rearrange("(m p) n -> p m n", p=P)  # [128, MT, OUT_DIM]

    with tile.TileContext(nc) as tc:
        loop_ctx = (
            tc.For_i(0, loop_n, 1,
                     hint_engines=(mybir.EngineType.PE, mybir.EngineType.SP,
                                   mybir.EngineType.DVE))
            if loop_n > 1 else contextlib.nullcontext()
        )
        with (
            loop_ctx,
            tc.tile_pool(name="xpool", bufs=XCHUNKS) as xpool,
            tc.tile_pool(name="wpool", bufs=int(__import__("os").environ.get("KWB", "3"))) as wpool,
            tc.tile_pool(name="opool", bufs=2) as opool,
            tc.tile_pool(name="pspool", bufs=PS_BUFS, space="PSUM") as pspool,
        ):
            x_chunks = []
            for c in range(XCHUNKS):
                x_c = xpool.tile([P, KPC * M_SHARD], dt_in,
                                 name=f"x{c}", tag="x")
                if do_in_dma:
                    nc.sync.dma_start(
                        out=x_c[:],
                        in_=xt[:, c * KPC * M_SHARD:(c + 1) * KPC * M_SHARD],
                    )
                else:
                    nc.vector.memset(x_c[:, 0:1], 0.0)
                x_chunks.append(x_c)

            shared_psums = None
            if not do_evict:
                shared_psums = [
                    pspool.tile([P, NB], mybir.dt.float32,
                                name=f"sps{m}", tag="ps")
                    for m in range(MT)
                ]
            for b in range(NBLK):
                w_b = wpool.tile([P, KT * NB], dt_in, name=f"w{b}", tag="w")
                if do_in_dma:
                    nc.sync.dma_start(
                        out=w_b[:],
                        in_=w[:, b * KT * NB:(b + 1) * KT * NB],
                    )
                else:
                    nc.vector.memset(w_b[:, 0:1], 0.0)

                if not do_compute:
                    continue
                o_b = None
                if do_evict:
                    o_b = opool.tile([P, MT, NB], mybir.dt.float32,
                                     name=f"o{b}", tag="o")
                if ORDER == "mi":
                    if shared_psums is not None:
                        psums = shared_psums
                    else:
                        psums = [
                            pspool.tile([P, NB], mybir.dt.float32,
                                        name=f"ps{b}_{m}", tag="ps")
                            for m in range(MT)
                        ]
                    for k in range(KT):
                        xc = x_chunks[k // KPC]
                        koff = (k % KPC) * M_SHARD
                        for m in range(MT):
                            nc.tensor.matmul(
                                psums[m][:],
                                xc[:, koff + m * P:koff + (m + 1) * P],
                                w_b[:, k * NB:(k + 1) * NB],
                                start=(k == 0),
                                stop=(k == KT - 1),
                            )
                    if do_evict:
                        for m in range(MT):
                            nc.vector.tensor_copy(o_b[:, m, :], psums[m][:])
                else:  # "ki": bank-dwell — one psum bank through all k
                    for m in range(MT):
                        ps = pspool.tile([P, NB], mybir.dt.float32,
                                         name=f"ps{b}_{m}", tag="ps")
                        for k in range(KT):
                            xc = x_chunks[k // KPC]
                            koff = (k % KPC) * M_SHARD
                            nc.tensor.matmul(
                                ps[:],
                                xc[:, koff + m * P:koff + (m + 1) * P],
                                w_b[:, k * NB:(k + 1) * NB],
                                start=(k == 0),
                                stop=(k == KT - 1),
                            )
                        nc.vector.tensor_copy(o_b[:, m, :], ps[:])
                if do_evict:
                    nc.scalar.dma_start(
                        out=out_r[:, :, b * NB:(b + 1) * NB],
                        in_=o_b[:],
                    )

    nc.compile()
    return nc


def _get_nc():
    global _cached_nc
    if _cached_nc is None:
        _cached_nc = _build()
    return _cached_nc


def _pack_x_shard(x_shard):
    """[M_SHARD, IN_DIM] -> [128, KT*M_SHARD] partition-major."""
    # dest[p, k*M_SHARD + m] = x_shard[m, k*128 + p]
    return np.ascontiguousarray(
        x_shard.T.reshape(KT, P, M_SHARD).transpose(1, 0, 2).reshape(P, -1)
    )


def _pack_w(w_full):
    """[IN_DIM, OUT_DIM] -> [128, NBLK*KT*NB] partition-major."""
    # dest[p, b*(KT*NB) + k*NB + j] = w_full[k*128 + p, b*NB + j]
    return np.ascontiguousarray(
        w_full.reshape(KT, P, NBLK, NB).transpose(1, 2, 0, 3).reshape(P, -1)
    )


def _make_in_maps(x, rules_outcome):
    np_dt = _np_dt()
    x = np.asarray(x, dtype=np_dt)
    w = np.asarray(rules_outcome, dtype=np_dt)
    assert x.shape == (BATCH, IN_DIM) and w.shape == (IN_DIM, OUT_DIM)
    w_packed = _pack_w(w)
    return [
        {
            "xt": _pack_x_shard(x[i * M_SHARD:(i + 1) * M_SHARD, :]),
            "w": w_packed,
        }
        for i in range(N_CORES)
    ]


def _run(x, rules_outcome, **spmd_kwargs):
    from concourse.bass_utils import run_bass_kernel_spmd

    in_maps = _make_in_maps(x, rules_outcome)
    nc = _get_nc()
    res = run_bass_kernel_spmd(nc, in_maps, core_ids=list(range(N_CORES)),
                               **spmd_kwargs)
    full = np.concatenate([res.results[i]["out"] for i in range(N_CORES)],
                          axis=0)
    return full, res


def kernel(x, rules_outcome):
    out, _ = _run(x, rules_outcome)
    return out
